# revision 22
# baseline (speedup 1.0000x reference)
"""BandSplit kernel for Trainium2 (8 NeuronCores, batch-parallel), fp16 I/O.

Math (per band i with offset off, width b, K = 2b):
  x[t,k]   : band slice of X, k = re/im-interleaved bins (reordered k = (c,f))
  z = ((x-mu)*rsqrt(var+eps)*gamma + beta) @ W + bias
    = rsqrt[t] * ( x @ Wg  +  mu[t]*(-colsum)  +  sigma[t]*cvec )
  with Wg = gamma*W (rows), colsum = sum_k Wg[k,:], cvec = beta@W + bias[i],
  sigma = sqrt(var+eps), rsqrt = 1/sigma.

All HBM I/O is fp16 (tolerance 2e-2; fp16 keeps rel err ~1e-3):
  X reordered on the host into k-major rows (no on-chip compaction), W
  augmented+reordered on the host, OUT written fp16 and upcast on the host.

Per band, the x rows live in ONE SBUF tile laid out as column blocks of
1024 t-columns; each matmul chunk (K<=128) reads partitions [0:K) of one
block, so a single DMA loads the whole band. mu/sigma rows are folded into
reserved partitions of the tile by a small partition-shift DMA.

Per core: batch element = core index. No collectives.
"""
import sys

sys.path.insert(0, "/opt/trn_rl_repo")
import numpy as np

BAND_BINS = [8] * 8 + [16] * 8 + [32] * 8 + [64] * 4 + [128] * 2 + [65]
NB = len(BAND_BINS)  # 31
D = 512
T = 1024
F = sum(BAND_BINS)  # 1025
EPS = 1e-5
NCORES = 8
NJ = T // 128  # 8 t-chunks


def plan():
    """Per-band layout. Returns list of dicts:
      off, b        : band position
      nxb           : number of 1024-col x blocks in the X tile
      nwb           : number of 512-col blocks in the W tile
      p_x           : partition rows of the X tile
      wrows         : block height of the W tile (equal for all its blocks)
      xr0           : starting row of this band in the reordered X HBM array
      wr0           : starting row of this band in the W HBM array
      xdma_rows     : rows of X HBM loaded (c-major), packed p=xdma_rows//c
      xchunks       : [(blk, k)] x-row chunks for stats (partitions [0:k))
      mains         : [(xblk, wblk, K)] main-matmul chunks
      ms            : (row, colblk) where mu/sigma rows live in the X tile
      sq            : (rows, cols) region to square for stats
    """
    bands = []
    xr = 0
    wr = 0
    for b in BAND_BINS:
        d = dict(b=b, xr0=xr, wr0=wr)
        if b <= 32:
            d.update(nxb=1, nwb=1, p_x=2 * b + 2, wrows=2 * b + 2,
                     xdma_rows=2 * b, xdma_p=2 * b,
                     xchunks=[(0, 2 * b)],
                     mains=[(0, 0, 2 * b + 2)],
                     ms=(2 * b, 0), sq=(2 * b, 1024))
        elif b == 64:
            d.update(nxb=2, nwb=2, p_x=66, wrows=66,
                     xdma_rows=128, xdma_p=64,
                     xchunks=[(0, 64), (1, 64)],
                     mains=[(0, 0, 64), (1, 1, 66)],
                     ms=(64, 1), sq=(64, 2048))
        elif b == 128:
            d.update(nxb=3, nwb=3, p_x=128, wrows=128,
                     xdma_rows=256, xdma_p=128,
                     xchunks=[(0, 128), (1, 128)],
                     mains=[(0, 0, 128), (1, 1, 128), (2, 2, 2)],
                     ms=(0, 2), sq=(128, 2048))
        else:  # b == 65
            d.update(nxb=2, nwb=2, p_x=67, wrows=67,
                     xdma_rows=130, xdma_p=65,
                     xchunks=[(0, 65), (1, 65)],
                     mains=[(0, 0, 65), (1, 1, 67)],
                     ms=(65, 1), sq=(65, 2048))
        d["off"] = sum(BAND_BINS[:len(bands)])
        xr += d["xdma_rows"]
        wr += d["wrows"] * d["nwb"]
        bands.append(d)
    return bands, xr, wr


BANDS, X_ROWS, W_ROWS = plan()  # X_ROWS == 2050


def build_x_perm():
    """Row permutation: X HBM row order is (band; c; f)."""
    perm = np.empty(X_ROWS, dtype=np.int64)
    r = 0
    for bd in BANDS:
        off, b = bd["off"], bd["b"]
        for c in (0, 1):
            perm[r:r + b] = c * F + np.arange(off, off + b)
            r += b
    return perm


X_PERM = build_x_perm()


def build_inputs_host(X, gamma, beta, W, bias):
    """Host-side: reorder X to k-major fp16 rows and build the augmented,
    per-band-blocked fp16 weight matrix."""
    # X: [B, F, T, 2] f32 -> [B, 2*F, T] c-major rows -> per-band order
    Xr = np.moveaxis(X, 3, 1).reshape(X.shape[0], 2 * F, T)
    Xp = np.ascontiguousarray(Xr[:, X_PERM, :]).astype(np.float16)

    w_aug = np.zeros((W_ROWS, D), dtype=np.float32)
    wg = gamma[:, None] * W  # [2F, D]
    for i, bd in enumerate(BANDS):
        off, b = bd["off"], bd["b"]
        s2 = 2 * off
        kidx = np.empty(2 * b, dtype=np.int64)
        kidx[0:b] = s2 + 2 * np.arange(b)          # re rows (c=0)
        kidx[b:2 * b] = s2 + 2 * np.arange(b) + 1  # im rows (c=1)
        xw = wg[kidx]  # [2b, D] in (c, f) order
        colsum = xw.sum(axis=0)
        cvec = beta[s2:s2 + 2 * b] @ W[s2:s2 + 2 * b] + bias[i]
        wr0, h = bd["wr0"], bd["wrows"]
        if bd["nwb"] == 1:
            w_aug[wr0:wr0 + 2 * b] = xw
            w_aug[wr0 + 2 * b] = -colsum
            w_aug[wr0 + 2 * b + 1] = cvec
        elif b in (64, 65):
            w_aug[wr0:wr0 + b] = xw[0:b]                 # blk0: re rows (+pad)
            w_aug[wr0 + h:wr0 + h + b] = xw[b:2 * b]     # blk1: im rows
            w_aug[wr0 + h + b] = -colsum
            w_aug[wr0 + h + b + 1] = cvec
        else:  # b == 128
            w_aug[wr0:wr0 + 128] = xw[0:128]
            w_aug[wr0 + 128:wr0 + 256] = xw[128:256]
            w_aug[wr0 + 256] = -colsum
            w_aug[wr0 + 257] = cvec
    return Xp, w_aug.astype(np.float16)


# PSUM->SBUF scaled-copy engine split: early bands DVE/Act weighted; late
# bands also use Pool (its SWDGE input-load work is all front-loaded).
DVE_COPY_FRAC = 0.25          # backs emitted before POOL_COPY_START
POOL_COPY_START = 7
LATE_PATTERN = "PADAADAP"     # per-j engine for backs >= POOL_COPY_START
PIPE_DEPTH = 4


def build_order():
    """Processing order: two small bands first (fast pipeline fill), then the
    7 compute-heavy bands (b>=64) spread evenly among the remaining smalls so
    per-band PE time stays below the output-DMA service rate."""
    smalls = list(range(24))
    bigs = [28, 29, 30, 24, 25, 26, 27]
    order = smalls[:2]
    si, bi = 2, 0
    while si < 24 or bi < 7:
        if bi < 7 and (si >= 24 or (bi + 1) * 22 <= (si - 1) * 7):
            order.append(bigs[bi])
            bi += 1
        else:
            order.append(smalls[si])
            si += 1
    return order


ORDER = build_order()


def build_nc():
    import concourse.bacc as bacc
    import concourse.tile as tile
    from concourse import mybir
    from concourse.masks import make_identity

    f32, f16 = mybir.dt.float32, mybir.dt.float16
    nc = bacc.Bacc(None)
    XH = nc.declare_dram_parameter("XP", [X_ROWS, T], f16, isOutput=False)
    WH = nc.declare_dram_parameter("WA", [W_ROWS, D], f16, isOutput=False)
    OUT = nc.declare_dram_parameter("OUT", [NB, T, D], f16, isOutput=True)

    with tile.TileContext(nc) as tc:
        with tc.tile_pool(name="consts", bufs=1) as consts, \
             tc.tile_pool(name="xps", bufs=24) as xps, \
             tc.tile_pool(name="xpm", bufs=5) as xpm, \
             tc.tile_pool(name="xpb", bufs=2) as xpb, \
             tc.tile_pool(name="wps", bufs=24) as wps, \
             tc.tile_pool(name="wpm", bufs=5) as wpm, \
             tc.tile_pool(name="wpb", bufs=2) as wpb, \
             tc.tile_pool(name="x2", bufs=4) as x2p, \
             tc.tile_pool(name="stat", bufs=8) as statp, \
             tc.tile_pool(name="stage", bufs=4) as stagep, \
             tc.tile_pool(name="pso", bufs=4, space="PSUM") as psop, \
             tc.tile_pool(name="pss", bufs=2, space="PSUM") as pssp, \
             tc.tile_pool(name="psm", bufs=2, space="PSUM") as psmp:

            Copy = mybir.ActivationFunctionType.Copy
            ident = consts.tile([128, 128], f32)
            make_identity(nc, ident)
            ones = consts.tile([128, 2], f16)
            nc.vector.memset(ones, 1.0)
            epsc = consts.tile([128, 1], f32)
            nc.vector.memset(epsc, EPS)

            # ---- upfront prefetch of all inputs on the GpSimd (SWDGE) queue;
            # pool bufs throttle how far ahead the loads actually run.
            xts, wts = {}, {}
            for bi in ORDER:
                bd = BANDS[bi]
                if bd["b"] <= 32:
                    xpool = xps
                elif bd["b"] == 128:
                    xpool = xpb
                else:
                    xpool = xpm
                xt = xpool.tile([bd["p_x"], bd["nxb"] * T], f16, tag="xt")
                xsrc = XH[bd["xr0"]:bd["xr0"] + bd["xdma_rows"], :]
                if bd["nxb"] == 1:
                    nc.gpsimd.dma_start(out=xt[0:bd["xdma_rows"], :], in_=xsrc)
                else:
                    nc.gpsimd.dma_start(
                        out=xt[0:bd["xdma_p"], 0:2 * T].rearrange(
                            "p (c t) -> p c t", c=2),
                        in_=xsrc.rearrange("(c p) t -> p c t", c=2))
                xts[bi] = xt

            def emit_front(i):
                """W load, square, stats matmuls, mu/sigma fold for band i"""
                bd = BANDS[i]
                b = bd["b"]
                inv_k = 1.0 / (2 * b)
                xt = xts[i]
                sqr, sqc = bd["sq"]

                # W load on the SP (HWDGE) queue, a few bands ahead of use
                wpool = wps if b <= 32 else (wpb if b == 128 else wpm)
                wt = wpool.tile([bd["wrows"], bd["nwb"] * D], f16, tag="wt")
                rows = bd["wrows"] * bd["nwb"]
                wsrc = WH[bd["wr0"]:bd["wr0"] + rows, :]
                if bd["nwb"] == 1:
                    nc.sync.dma_start(out=wt[:, :], in_=wsrc)
                else:
                    nc.sync.dma_start(
                        out=wt[:, :].rearrange("p (c d) -> p c d", c=bd["nwb"]),
                        in_=wsrc.rearrange("(c p) d -> p c d", c=bd["nwb"]))
                wts[i] = wt

                x2 = x2p.tile([sqr, sqc], f16, tag="x2")
                nc.vector.tensor_mul(x2, xt[0:sqr, 0:sqc], xt[0:sqr, 0:sqc])

                xchunks = bd["xchunks"]
                last_x = len(xchunks) - 1
                pc = pssp.tile([128, 32], f32, tag="pc")
                for j in range(NJ):
                    for xi, (blk, k) in enumerate(xchunks):
                        c0 = blk * T + j * 128
                        nc.tensor.matmul(pc[:, 2 * j:2 * j + 2],
                                         xt[0:k, c0:c0 + 128],
                                         ones[0:k, :],
                                         start=(xi == 0), stop=(xi == last_x))
                for j in range(NJ):
                    for xi, (blk, k) in enumerate(xchunks):
                        c0 = blk * T + j * 128
                        nc.tensor.matmul(pc[:, 16 + 2 * j:18 + 2 * j],
                                         x2[0:k, c0:c0 + 128],
                                         ones[0:k, :],
                                         start=(xi == 0), stop=(xi == last_x))

                # batched stats processing; ms = [mu cols | sigma cols]
                ms = statp.tile([128, 16], f32, tag="ms")
                rs = statp.tile([128, NJ], f32, tag="rs")
                tmpe = statp.tile([128, NJ], f32, tag="tmpe")
                tmpm = statp.tile([128, NJ], f32, tag="tmpm")
                pcx = pc[:, 0:16].rearrange("p (a c) -> p c a", c=2)[:, 0, :]
                pcx2 = pc[:, 16:32].rearrange("p (a c) -> p c a", c=2)[:, 0, :]
                nc.vector.tensor_scalar_mul(ms[:, 0:8], pcx, inv_k)    # mu
                nc.vector.tensor_scalar_mul(tmpe, pcx2, inv_k)         # E[x^2]
                nc.vector.tensor_mul(tmpm, ms[:, 0:8], ms[:, 0:8])     # mu^2
                nc.vector.tensor_sub(tmpe, tmpe, tmpm)                 # var
                nc.scalar.activation(out=ms[:, 8:16], in_=tmpe,
                                     func=mybir.ActivationFunctionType.Sqrt,
                                     bias=epsc, scale=1.0)             # sigma
                nc.vector.reciprocal(out=rs, in_=ms[:, 8:16])          # rsqrt
                return dict(i=i, rs=rs, ms=ms)

            def emit_mid(stt):
                """mu/sigma rows via PE transpose + partition-fold DMA.
                Emitted well after front(i) so the PE transpose (which waits
                on the DVE/Act stats chain) never blocks later bands' stats
                matmuls in the in-order PE queue."""
                i, ms = stt["i"], stt["ms"]
                bd = BANDS[i]
                xt = xts[i]
                mt = psmp.tile([16, 128], f32, tag="mt")
                nc.tensor.transpose(mt, ms, ident)
                mts = statp.tile([16, 128], f16, tag="mts")
                nc.vector.tensor_scalar_mul(mts, mt, 1.0)
                mrow, mblk = bd["ms"]
                nc.sync.dma_start(
                    out=xt[mrow:mrow + 2, mblk * T:(mblk + 1) * T]
                    .rearrange("r (j p) -> r j p", j=NJ),
                    in_=mts[:, :])

            copy_acc = [0.0]
            nback = [0]

            def emit_back(stt):
                """main matmuls + scale-copy + out DMA for band stt['i']"""
                i, rs = stt["i"], stt["rs"]
                bd = BANDS[i]
                xt, wt = xts[i], wts[i]
                mains = bd["mains"]
                stage = stagep.tile([128, NJ, D], f16, tag="stage")
                for j in range(NJ):
                    po = psop.tile([128, D], f32, tag="po")
                    for ci, (xblk, wblk, K) in enumerate(mains):
                        nc.tensor.matmul(
                            po, xt[0:K, xblk * T + j * 128:xblk * T + (j + 1) * 128],
                            wt[0:K, wblk * D:(wblk + 1) * D],
                            start=(ci == 0), stop=(ci == len(mains) - 1))
                    # engine split of the PSUM->SBUF scaled copies
                    if nback[0] >= POOL_COPY_START:
                        eng = LATE_PATTERN[j]
                    else:
                        copy_acc[0] += DVE_COPY_FRAC
                        if copy_acc[0] >= 1.0:
                            copy_acc[0] -= 1.0
                            eng = "D"
                        else:
                            eng = "A"
                    if eng == "D":
                        nc.vector.tensor_scalar_mul(stage[:, j, :], po,
                                                    rs[:, j:j + 1])
                    elif eng == "P":
                        nc.gpsimd.tensor_scalar_mul(stage[:, j, :], po,
                                                    rs[:, j:j + 1])
                    else:
                        nc.scalar.activation(out=stage[:, j, :], in_=po,
                                             func=Copy, scale=rs[:, j:j + 1])
                nc.sync.dma_start(
                    out=OUT[i, :, :].rearrange("(j p) d -> p j d", p=128),
                    in_=stage)
                nback[0] += 1

            # ---- software pipeline: front(i) runs ahead; mid(i) one band
            # ahead of back(i); depth ramps 1 -> PIPE_DEPTH for fast start.
            from collections import deque
            pend = deque()
            midq = deque()
            for idx, i in enumerate(ORDER):
                pend.append(emit_front(i))
                depth = 1 if idx < 2 else PIPE_DEPTH
                while len(pend) > depth:
                    s = pend.popleft()
                    emit_mid(s)
                    midq.append(s)
                    if len(midq) > 1:
                        emit_back(midq.popleft())
            while pend:
                s = pend.popleft()
                emit_mid(s)
                midq.append(s)
                if len(midq) > 1:
                    emit_back(midq.popleft())
            while midq:
                emit_back(midq.popleft())

    nc.finalize()
    return nc


_NC = None


def kernel(X, gamma, beta, W, bias):
    global _NC
    from concourse.bass_utils import run_bass_kernel_spmd

    X = np.asarray(X, dtype=np.float32)
    gamma = np.asarray(gamma, dtype=np.float32)
    beta = np.asarray(beta, dtype=np.float32)
    W = np.asarray(W, dtype=np.float32)
    bias = np.asarray(bias, dtype=np.float32)

    Xp, w_aug = build_inputs_host(X, gamma, beta, W, bias)
    if _NC is None:
        _NC = build_nc()
    in_maps = [{"XP": Xp[b], "WA": w_aug} for b in range(NCORES)]
    res = run_bass_kernel_spmd(_NC, in_maps, list(range(NCORES))).results
    return np.stack([res[b]["OUT"] for b in range(NCORES)], axis=0).astype(
        np.float32)


# revision 23
# speedup vs baseline: 1.2017x; 1.2017x over previous
"""BandSplit kernel for Trainium2 (8 NeuronCores, batch-parallel), fp16 I/O.

Math (per band i with offset off, width b, K = 2b):
  x[t,k]   : band slice of X, k = re/im-interleaved bins (reordered k = (c,f))
  z = ((x-mu)*rsqrt(var+eps)*gamma + beta) @ W + bias
    = rsqrt[t] * ( x @ Wg  +  mu[t]*(-colsum)  +  sigma[t]*cvec )
  with Wg = gamma*W (rows), colsum = sum_k Wg[k,:], cvec = beta@W + bias[i],
  sigma = sqrt(var+eps), rsqrt = 1/sigma.

All HBM I/O is fp16 (tolerance 2e-2; fp16 keeps rel err ~1e-3):
  X reordered on the host into k-major rows (no on-chip compaction), W
  augmented+reordered on the host, OUT written fp16 and upcast on the host.

Per band, the x rows live in ONE SBUF tile laid out as column blocks of
1024 t-columns; each matmul chunk (K<=128) reads partitions [0:K) of one
block, so a single DMA loads the whole band. mu/sigma rows are folded into
reserved partitions of the tile by a small partition-shift DMA.

Per core: batch element = core index. No collectives.
"""
import sys

sys.path.insert(0, "/opt/trn_rl_repo")
import numpy as np

BAND_BINS = [8] * 8 + [16] * 8 + [32] * 8 + [64] * 4 + [128] * 2 + [65]
NB = len(BAND_BINS)  # 31
D = 512
T = 1024
F = sum(BAND_BINS)  # 1025
EPS = 1e-5
NCORES = 8
NJ = T // 128  # 8 t-chunks


def plan():
    """Per-band layout. Returns list of dicts:
      off, b        : band position
      nxb           : number of 1024-col x blocks in the X tile
      nwb           : number of 512-col blocks in the W tile
      p_x           : partition rows of the X tile
      wrows         : block height of the W tile (equal for all its blocks)
      xr0           : starting row of this band in the reordered X HBM array
      wr0           : starting row of this band in the W HBM array
      xdma_rows     : rows of X HBM loaded (c-major), packed p=xdma_rows//c
      xchunks       : [(blk, k)] x-row chunks for stats (partitions [0:k))
      mains         : [(xblk, wblk, K)] main-matmul chunks
      ms            : (row, colblk) where mu/sigma rows live in the X tile
      sq            : (rows, cols) region to square for stats
    """
    bands = []
    xr = 0
    wr = 0
    for b in BAND_BINS:
        d = dict(b=b, xr0=xr, wr0=wr)
        if b <= 32:
            d.update(nxb=1, nwb=1, p_x=2 * b + 2, wrows=2 * b + 2,
                     xdma_rows=2 * b, xdma_p=2 * b,
                     xchunks=[(0, 2 * b)],
                     mains=[(0, 0, 2 * b + 2)],
                     ms=(2 * b, 0), sq=(2 * b, 1024))
        elif b == 64:
            d.update(nxb=2, nwb=2, p_x=66, wrows=66,
                     xdma_rows=128, xdma_p=64,
                     xchunks=[(0, 64), (1, 64)],
                     mains=[(0, 0, 64), (1, 1, 66)],
                     ms=(64, 1), sq=(64, 2048))
        elif b == 128:
            d.update(nxb=3, nwb=3, p_x=128, wrows=128,
                     xdma_rows=256, xdma_p=128,
                     xchunks=[(0, 128), (1, 128)],
                     mains=[(0, 0, 128), (1, 1, 128), (2, 2, 2)],
                     ms=(0, 2), sq=(128, 2048))
        else:  # b == 65
            d.update(nxb=2, nwb=2, p_x=67, wrows=67,
                     xdma_rows=130, xdma_p=65,
                     xchunks=[(0, 65), (1, 65)],
                     mains=[(0, 0, 65), (1, 1, 67)],
                     ms=(65, 1), sq=(65, 2048))
        d["off"] = sum(BAND_BINS[:len(bands)])
        xr += d["xdma_rows"]
        wr += d["wrows"] * d["nwb"]
        bands.append(d)
    return bands, xr, wr


BANDS, X_ROWS, W_ROWS = plan()  # X_ROWS == 2050


def build_x_perm():
    """Row permutation: X HBM row order is (band; c; f)."""
    perm = np.empty(X_ROWS, dtype=np.int64)
    r = 0
    for bd in BANDS:
        off, b = bd["off"], bd["b"]
        for c in (0, 1):
            perm[r:r + b] = c * F + np.arange(off, off + b)
            r += b
    return perm


X_PERM = build_x_perm()


def build_inputs_host(X, gamma, beta, W, bias):
    """Host-side: reorder X to k-major fp16 rows and build the augmented,
    per-band-blocked fp16 weight matrix."""
    # X: [B, F, T, 2] f32 -> [B, 2*F, T] c-major rows -> per-band order
    Xr = np.moveaxis(X, 3, 1).reshape(X.shape[0], 2 * F, T)
    Xp = np.ascontiguousarray(Xr[:, X_PERM, :]).astype(np.float16)

    w_aug = np.zeros((W_ROWS, D), dtype=np.float32)
    wg = gamma[:, None] * W  # [2F, D]
    for i, bd in enumerate(BANDS):
        off, b = bd["off"], bd["b"]
        s2 = 2 * off
        kidx = np.empty(2 * b, dtype=np.int64)
        kidx[0:b] = s2 + 2 * np.arange(b)          # re rows (c=0)
        kidx[b:2 * b] = s2 + 2 * np.arange(b) + 1  # im rows (c=1)
        xw = wg[kidx]  # [2b, D] in (c, f) order
        colsum = xw.sum(axis=0)
        cvec = beta[s2:s2 + 2 * b] @ W[s2:s2 + 2 * b] + bias[i]
        wr0, h = bd["wr0"], bd["wrows"]
        if bd["nwb"] == 1:
            w_aug[wr0:wr0 + 2 * b] = xw
            w_aug[wr0 + 2 * b] = -colsum
            w_aug[wr0 + 2 * b + 1] = cvec
        elif b in (64, 65):
            w_aug[wr0:wr0 + b] = xw[0:b]                 # blk0: re rows (+pad)
            w_aug[wr0 + h:wr0 + h + b] = xw[b:2 * b]     # blk1: im rows
            w_aug[wr0 + h + b] = -colsum
            w_aug[wr0 + h + b + 1] = cvec
        else:  # b == 128
            w_aug[wr0:wr0 + 128] = xw[0:128]
            w_aug[wr0 + 128:wr0 + 256] = xw[128:256]
            w_aug[wr0 + 256] = -colsum
            w_aug[wr0 + 257] = cvec
    return Xp, w_aug.astype(np.float16)


# PSUM->SBUF scaled-copy engine split: early bands DVE/Act weighted; late
# bands also use Pool (its SWDGE input-load work is all front-loaded).
DVE_COPY_FRAC = 0.25          # backs emitted before POOL_COPY_START
POOL_COPY_START = 7
LATE_PATTERN = "PADAADAP"     # per-j engine for backs >= POOL_COPY_START
PIPE_DEPTH = 4


def build_order():
    """Processing order: two small bands first (fast pipeline fill), then the
    7 compute-heavy bands (b>=64) spread evenly among the remaining smalls so
    per-band PE time stays below the output-DMA service rate."""
    smalls = list(range(24))
    bigs = [28, 29, 30, 24, 25, 26, 27]
    order = smalls[:2]
    si, bi = 2, 0
    while si < 24 or bi < 7:
        if bi < 7 and (si >= 24 or (bi + 1) * 22 <= (si - 1) * 7):
            order.append(bigs[bi])
            bi += 1
        else:
            order.append(smalls[si])
            si += 1
    return order


ORDER = build_order()


def build_nc():
    import concourse.bacc as bacc
    import concourse.tile as tile
    from concourse import mybir
    from concourse.masks import make_identity

    f32, f16 = mybir.dt.float32, mybir.dt.float16
    nc = bacc.Bacc(None)
    XH = nc.declare_dram_parameter("XP", [X_ROWS, T], f16, isOutput=False)
    WH = nc.declare_dram_parameter("WA", [W_ROWS, D], f16, isOutput=False)
    OUT = nc.declare_dram_parameter("OUT", [NB, T, D], f16, isOutput=True)

    with tile.TileContext(nc) as tc:
        with tc.tile_pool(name="consts", bufs=1) as consts, \
             tc.tile_pool(name="xps", bufs=24) as xps, \
             tc.tile_pool(name="xpm", bufs=5) as xpm, \
             tc.tile_pool(name="xpb", bufs=2) as xpb, \
             tc.tile_pool(name="wps", bufs=24) as wps, \
             tc.tile_pool(name="wpm", bufs=5) as wpm, \
             tc.tile_pool(name="wpb", bufs=2) as wpb, \
             tc.tile_pool(name="x2", bufs=4) as x2p, \
             tc.tile_pool(name="stat", bufs=12) as statp, \
             tc.tile_pool(name="stage", bufs=4) as stagep, \
             tc.tile_pool(name="pso", bufs=4, space="PSUM") as psop, \
             tc.tile_pool(name="pss", bufs=2, space="PSUM") as pssp, \
             tc.tile_pool(name="psm", bufs=2, space="PSUM") as psmp:

            Copy = mybir.ActivationFunctionType.Copy
            ident = consts.tile([128, 128], f32)
            make_identity(nc, ident)
            ones = consts.tile([128, 2], f16)
            nc.vector.memset(ones, 1.0)
            epsc = consts.tile([128, 1], f32)
            nc.vector.memset(epsc, EPS)

            # ---- upfront prefetch of all inputs on the GpSimd (SWDGE) queue;
            # pool bufs throttle how far ahead the loads actually run.
            xts, wts = {}, {}
            for bi in ORDER:
                bd = BANDS[bi]
                if bd["b"] <= 32:
                    xpool = xps
                elif bd["b"] == 128:
                    xpool = xpb
                else:
                    xpool = xpm
                xt = xpool.tile([bd["p_x"], bd["nxb"] * T], f16, tag="xt")
                xsrc = XH[bd["xr0"]:bd["xr0"] + bd["xdma_rows"], :]
                if bd["nxb"] == 1:
                    nc.gpsimd.dma_start(out=xt[0:bd["xdma_rows"], :], in_=xsrc)
                else:
                    nc.gpsimd.dma_start(
                        out=xt[0:bd["xdma_p"], 0:2 * T].rearrange(
                            "p (c t) -> p c t", c=2),
                        in_=xsrc.rearrange("(c p) t -> p c t", c=2))
                xts[bi] = xt

            def emit_front(i):
                """W load, square, stats matmuls, mu/sigma fold for band i"""
                bd = BANDS[i]
                b = bd["b"]
                inv_k = 1.0 / (2 * b)
                xt = xts[i]
                sqr, sqc = bd["sq"]

                # W load on the SP (HWDGE) queue, a few bands ahead of use
                wpool = wps if b <= 32 else (wpb if b == 128 else wpm)
                wt = wpool.tile([bd["wrows"], bd["nwb"] * D], f16, tag="wt")
                rows = bd["wrows"] * bd["nwb"]
                wsrc = WH[bd["wr0"]:bd["wr0"] + rows, :]
                if bd["nwb"] == 1:
                    nc.sync.dma_start(out=wt[:, :], in_=wsrc)
                else:
                    nc.sync.dma_start(
                        out=wt[:, :].rearrange("p (c d) -> p c d", c=bd["nwb"]),
                        in_=wsrc.rearrange("(c p) d -> p c d", c=bd["nwb"]))
                wts[i] = wt

                x2 = x2p.tile([sqr, sqc], f16, tag="x2")
                nc.vector.tensor_mul(x2, xt[0:sqr, 0:sqc], xt[0:sqr, 0:sqc])

                xchunks = bd["xchunks"]
                last_x = len(xchunks) - 1
                pc = pssp.tile([128, 32], f32, tag="pc")
                for j in range(NJ):
                    for xi, (blk, k) in enumerate(xchunks):
                        c0 = blk * T + j * 128
                        nc.tensor.matmul(pc[:, 2 * j:2 * j + 2],
                                         xt[0:k, c0:c0 + 128],
                                         ones[0:k, :],
                                         start=(xi == 0), stop=(xi == last_x))
                for j in range(NJ):
                    for xi, (blk, k) in enumerate(xchunks):
                        c0 = blk * T + j * 128
                        nc.tensor.matmul(pc[:, 16 + 2 * j:18 + 2 * j],
                                         x2[0:k, c0:c0 + 128],
                                         ones[0:k, :],
                                         start=(xi == 0), stop=(xi == last_x))

                # batched stats processing; ms = [mu cols | sigma cols]
                ms = statp.tile([128, 16], f32, tag="ms")
                rs = statp.tile([128, NJ], f32, tag="rs")
                tmpe = statp.tile([128, NJ], f32, tag="tmpe")
                tmpm = statp.tile([128, NJ], f32, tag="tmpm")
                pcx = pc[:, 0:16].rearrange("p (a c) -> p c a", c=2)[:, 0, :]
                pcx2 = pc[:, 16:32].rearrange("p (a c) -> p c a", c=2)[:, 0, :]
                nc.vector.tensor_scalar_mul(ms[:, 0:8], pcx, inv_k)    # mu
                nc.vector.tensor_scalar_mul(tmpe, pcx2, inv_k)         # E[x^2]
                nc.vector.tensor_mul(tmpm, ms[:, 0:8], ms[:, 0:8])     # mu^2
                nc.vector.tensor_sub(tmpe, tmpe, tmpm)                 # var
                nc.scalar.activation(out=ms[:, 8:16], in_=tmpe,
                                     func=mybir.ActivationFunctionType.Sqrt,
                                     bias=epsc, scale=1.0)             # sigma
                nc.vector.reciprocal(out=rs, in_=ms[:, 8:16])          # rsqrt
                return dict(i=i, rs=rs, ms=ms)

            def emit_mid(stt):
                """mu/sigma rows via PE transpose + partition-fold DMA.
                Emitted well after front(i) so the PE transpose (which waits
                on the DVE/Act stats chain) never blocks later bands' stats
                matmuls in the in-order PE queue."""
                i, ms = stt["i"], stt["ms"]
                bd = BANDS[i]
                xt = xts[i]
                mt = psmp.tile([16, 128], f32, tag="mt")
                nc.tensor.transpose(mt, ms, ident)
                mts = statp.tile([16, 128], f16, tag="mts")
                nc.vector.tensor_scalar_mul(mts, mt, 1.0)
                mrow, mblk = bd["ms"]
                nc.sync.dma_start(
                    out=xt[mrow:mrow + 2, mblk * T:(mblk + 1) * T]
                    .rearrange("r (j p) -> r j p", j=NJ),
                    in_=mts[:, :])

            copy_acc = [0.0]
            nback = [0]

            def emit_back(stt):
                """main matmuls + scale-copy + out DMA for band stt['i']"""
                i, rs = stt["i"], stt["rs"]
                bd = BANDS[i]
                xt, wt = xts[i], wts[i]
                mains = bd["mains"]
                stage = stagep.tile([128, NJ, D], f16, tag="stage")
                for j in range(NJ):
                    po = psop.tile([128, D], f32, tag="po")
                    for ci, (xblk, wblk, K) in enumerate(mains):
                        nc.tensor.matmul(
                            po, xt[0:K, xblk * T + j * 128:xblk * T + (j + 1) * 128],
                            wt[0:K, wblk * D:(wblk + 1) * D],
                            start=(ci == 0), stop=(ci == len(mains) - 1))
                    # engine split of the PSUM->SBUF scaled copies
                    if nback[0] >= POOL_COPY_START:
                        eng = LATE_PATTERN[j]
                    else:
                        copy_acc[0] += DVE_COPY_FRAC
                        if copy_acc[0] >= 1.0:
                            copy_acc[0] -= 1.0
                            eng = "D"
                        else:
                            eng = "A"
                    if eng == "D":
                        nc.vector.tensor_scalar_mul(stage[:, j, :], po,
                                                    rs[:, j:j + 1])
                    elif eng == "P":
                        nc.gpsimd.tensor_scalar_mul(stage[:, j, :], po,
                                                    rs[:, j:j + 1])
                    else:
                        nc.scalar.activation(out=stage[:, j, :], in_=po,
                                             func=Copy, scale=rs[:, j:j + 1])
                nc.sync.dma_start(
                    out=OUT[i, :, :].rearrange("(j p) d -> p j d", p=128),
                    in_=stage)
                nback[0] += 1

            # ---- software pipeline: front(i) runs ahead; mid(i) one band
            # ahead of back(i); depth ramps 1 -> PIPE_DEPTH for fast start.
            from collections import deque
            pend = deque()
            midq = deque()
            for idx, i in enumerate(ORDER):
                pend.append(emit_front(i))
                depth = 1 if idx < 2 else PIPE_DEPTH
                while len(pend) > depth:
                    s = pend.popleft()
                    emit_mid(s)
                    midq.append(s)
                    if len(midq) > 2:
                        emit_back(midq.popleft())
            while pend:
                s = pend.popleft()
                emit_mid(s)
                midq.append(s)
                if len(midq) > 2:
                    emit_back(midq.popleft())
            while midq:
                emit_back(midq.popleft())

    nc.finalize()
    return nc


_NC = None


def kernel(X, gamma, beta, W, bias):
    global _NC
    from concourse.bass_utils import run_bass_kernel_spmd

    X = np.asarray(X, dtype=np.float32)
    gamma = np.asarray(gamma, dtype=np.float32)
    beta = np.asarray(beta, dtype=np.float32)
    W = np.asarray(W, dtype=np.float32)
    bias = np.asarray(bias, dtype=np.float32)

    Xp, w_aug = build_inputs_host(X, gamma, beta, W, bias)
    if _NC is None:
        _NC = build_nc()
    in_maps = [{"XP": Xp[b], "WA": w_aug} for b in range(NCORES)]
    res = run_bass_kernel_spmd(_NC, in_maps, list(range(NCORES))).results
    return np.stack([res[b]["OUT"] for b in range(NCORES)], axis=0).astype(
        np.float32)


# revision 25
# speedup vs baseline: 1.2134x; 1.0097x over previous
"""BandSplit kernel for Trainium2 (8 NeuronCores, batch-parallel), fp16 I/O.

Math (per band i with offset off, width b, K = 2b):
  x[t,k]   : band slice of X, k = re/im-interleaved bins (reordered k = (c,f))
  z = ((x-mu)*rsqrt(var+eps)*gamma + beta) @ W + bias
    = rsqrt[t] * ( x @ Wg  +  mu[t]*(-colsum)  +  sigma[t]*cvec )
  with Wg = gamma*W (rows), colsum = sum_k Wg[k,:], cvec = beta@W + bias[i],
  sigma = sqrt(var+eps), rsqrt = 1/sigma.

All HBM I/O is fp16 (tolerance 2e-2; fp16 keeps rel err ~1e-3):
  X reordered on the host into k-major rows (no on-chip compaction), W
  augmented+reordered on the host, OUT written fp16 and upcast on the host.

Per band, the x rows live in ONE SBUF tile laid out as column blocks of
1024 t-columns; each matmul chunk (K<=128) reads partitions [0:K) of one
block, so a single DMA loads the whole band. mu/sigma rows are folded into
reserved partitions of the tile by a small partition-shift DMA.

Per core: batch element = core index. No collectives.
"""
import sys

sys.path.insert(0, "/opt/trn_rl_repo")
import numpy as np

BAND_BINS = [8] * 8 + [16] * 8 + [32] * 8 + [64] * 4 + [128] * 2 + [65]
NB = len(BAND_BINS)  # 31
D = 512
T = 1024
F = sum(BAND_BINS)  # 1025
EPS = 1e-5
NCORES = 8
NJ = T // 128  # 8 t-chunks


def plan():
    """Per-band layout. Returns list of dicts:
      off, b        : band position
      nxb           : number of 1024-col x blocks in the X tile
      nwb           : number of 512-col blocks in the W tile
      p_x           : partition rows of the X tile
      wrows         : block height of the W tile (equal for all its blocks)
      xr0           : starting row of this band in the reordered X HBM array
      wr0           : starting row of this band in the W HBM array
      xdma_rows     : rows of X HBM loaded (c-major), packed p=xdma_rows//c
      xchunks       : [(blk, k)] x-row chunks for stats (partitions [0:k))
      mains         : [(xblk, wblk, K)] main-matmul chunks
      ms            : (row, colblk) where mu/sigma rows live in the X tile
      sq            : (rows, cols) region to square for stats
    """
    bands = []
    xr = 0
    wr = 0
    for b in BAND_BINS:
        d = dict(b=b, xr0=xr, wr0=wr)
        if b <= 32:
            d.update(nxb=1, nwb=1, p_x=2 * b + 2, wrows=2 * b + 2,
                     xdma_rows=2 * b, xdma_p=2 * b,
                     xchunks=[(0, 2 * b)],
                     mains=[(0, 0, 2 * b + 2)],
                     ms=(2 * b, 0), sq=(2 * b, 1024))
        elif b == 64:
            d.update(nxb=2, nwb=2, p_x=66, wrows=66,
                     xdma_rows=128, xdma_p=64,
                     xchunks=[(0, 64), (1, 64)],
                     mains=[(0, 0, 64), (1, 1, 66)],
                     ms=(64, 1), sq=(64, 2048))
        elif b == 128:
            d.update(nxb=3, nwb=3, p_x=128, wrows=128,
                     xdma_rows=256, xdma_p=128,
                     xchunks=[(0, 128), (1, 128)],
                     mains=[(0, 0, 128), (1, 1, 128), (2, 2, 2)],
                     ms=(0, 2), sq=(128, 2048))
        else:  # b == 65
            d.update(nxb=2, nwb=2, p_x=67, wrows=67,
                     xdma_rows=130, xdma_p=65,
                     xchunks=[(0, 65), (1, 65)],
                     mains=[(0, 0, 65), (1, 1, 67)],
                     ms=(65, 1), sq=(65, 2048))
        d["off"] = sum(BAND_BINS[:len(bands)])
        xr += d["xdma_rows"]
        wr += d["wrows"] * d["nwb"]
        bands.append(d)
    return bands, xr, wr


BANDS, X_ROWS, W_ROWS = plan()  # X_ROWS == 2050


def build_x_perm():
    """Row permutation: X HBM row order is (band; c; f)."""
    perm = np.empty(X_ROWS, dtype=np.int64)
    r = 0
    for bd in BANDS:
        off, b = bd["off"], bd["b"]
        for c in (0, 1):
            perm[r:r + b] = c * F + np.arange(off, off + b)
            r += b
    return perm


X_PERM = build_x_perm()


def build_inputs_host(X, gamma, beta, W, bias):
    """Host-side: reorder X to k-major fp16 rows and build the augmented,
    per-band-blocked fp16 weight matrix."""
    # X: [B, F, T, 2] f32 -> [B, 2*F, T] c-major rows -> per-band order
    Xr = np.moveaxis(X, 3, 1).reshape(X.shape[0], 2 * F, T)
    Xp = np.ascontiguousarray(Xr[:, X_PERM, :]).astype(np.float16)

    w_aug = np.zeros((W_ROWS, D), dtype=np.float32)
    wg = gamma[:, None] * W  # [2F, D]
    for i, bd in enumerate(BANDS):
        off, b = bd["off"], bd["b"]
        s2 = 2 * off
        kidx = np.empty(2 * b, dtype=np.int64)
        kidx[0:b] = s2 + 2 * np.arange(b)          # re rows (c=0)
        kidx[b:2 * b] = s2 + 2 * np.arange(b) + 1  # im rows (c=1)
        xw = wg[kidx]  # [2b, D] in (c, f) order
        colsum = xw.sum(axis=0)
        cvec = beta[s2:s2 + 2 * b] @ W[s2:s2 + 2 * b] + bias[i]
        wr0, h = bd["wr0"], bd["wrows"]
        if bd["nwb"] == 1:
            w_aug[wr0:wr0 + 2 * b] = xw
            w_aug[wr0 + 2 * b] = -colsum
            w_aug[wr0 + 2 * b + 1] = cvec
        elif b in (64, 65):
            w_aug[wr0:wr0 + b] = xw[0:b]                 # blk0: re rows (+pad)
            w_aug[wr0 + h:wr0 + h + b] = xw[b:2 * b]     # blk1: im rows
            w_aug[wr0 + h + b] = -colsum
            w_aug[wr0 + h + b + 1] = cvec
        else:  # b == 128
            w_aug[wr0:wr0 + 128] = xw[0:128]
            w_aug[wr0 + 128:wr0 + 256] = xw[128:256]
            w_aug[wr0 + 256] = -colsum
            w_aug[wr0 + 257] = cvec
    return Xp, w_aug.astype(np.float16)


# PSUM->SBUF scaled-copy engine split: early bands DVE/Act weighted; late
# bands also use Pool (its SWDGE input-load work is all front-loaded).
DVE_COPY_FRAC = 0.25          # backs emitted before POOL_COPY_START
POOL_COPY_START = 7
LATE_PATTERN = "PADAADAP"     # per-j engine for backs >= POOL_COPY_START
PIPE_DEPTH = 4
import os
LAG_M = int(os.environ.get("LAG_M", "2"))
LAG_B = int(os.environ.get("LAG_B", "4"))


def build_order():
    """Processing order: two small bands first (fast pipeline fill), then the
    7 compute-heavy bands (b>=64) spread evenly among the remaining smalls so
    per-band PE time stays below the output-DMA service rate."""
    smalls = list(range(24))
    bigs = [28, 29, 30, 24, 25, 26, 27]
    order = smalls[:2]
    si, bi = 2, 0
    while si < 24 or bi < 7:
        if bi < 7 and (si >= 24 or (bi + 1) * 22 <= (si - 1) * 7):
            order.append(bigs[bi])
            bi += 1
        else:
            order.append(smalls[si])
            si += 1
    return order


ORDER = build_order()


def build_nc():
    import concourse.bacc as bacc
    import concourse.tile as tile
    from concourse import mybir
    from concourse.masks import make_identity

    f32, f16 = mybir.dt.float32, mybir.dt.float16
    nc = bacc.Bacc(None)
    XH = nc.declare_dram_parameter("XP", [X_ROWS, T], f16, isOutput=False)
    WH = nc.declare_dram_parameter("WA", [W_ROWS, D], f16, isOutput=False)
    OUT = nc.declare_dram_parameter("OUT", [NB, T, D], f16, isOutput=True)

    with tile.TileContext(nc) as tc:
        with tc.tile_pool(name="consts", bufs=1) as consts, \
             tc.tile_pool(name="xps", bufs=24) as xps, \
             tc.tile_pool(name="xpm", bufs=5) as xpm, \
             tc.tile_pool(name="xpb", bufs=2) as xpb, \
             tc.tile_pool(name="wps", bufs=24) as wps, \
             tc.tile_pool(name="wpm", bufs=5) as wpm, \
             tc.tile_pool(name="wpb", bufs=2) as wpb, \
             tc.tile_pool(name="x2", bufs=4) as x2p, \
             tc.tile_pool(name="stat", bufs=12) as statp, \
             tc.tile_pool(name="stage", bufs=4) as stagep, \
             tc.tile_pool(name="pso", bufs=4, space="PSUM") as psop, \
             tc.tile_pool(name="pss", bufs=2, space="PSUM") as pssp, \
             tc.tile_pool(name="psm", bufs=2, space="PSUM") as psmp:

            Copy = mybir.ActivationFunctionType.Copy
            ident = consts.tile([128, 128], f32)
            make_identity(nc, ident)
            ones = consts.tile([128, 2], f16)
            nc.vector.memset(ones, 1.0)
            epsc = consts.tile([128, 1], f32)
            nc.vector.memset(epsc, EPS)

            # ---- upfront prefetch of all inputs on the GpSimd (SWDGE) queue;
            # pool bufs throttle how far ahead the loads actually run.
            xts, wts = {}, {}
            for bi in ORDER:
                bd = BANDS[bi]
                if bd["b"] <= 32:
                    xpool = xps
                elif bd["b"] == 128:
                    xpool = xpb
                else:
                    xpool = xpm
                xt = xpool.tile([bd["p_x"], bd["nxb"] * T], f16, tag="xt")
                xsrc = XH[bd["xr0"]:bd["xr0"] + bd["xdma_rows"], :]
                if bd["nxb"] == 1:
                    nc.gpsimd.dma_start(out=xt[0:bd["xdma_rows"], :], in_=xsrc)
                else:
                    nc.gpsimd.dma_start(
                        out=xt[0:bd["xdma_p"], 0:2 * T].rearrange(
                            "p (c t) -> p c t", c=2),
                        in_=xsrc.rearrange("(c p) t -> p c t", c=2))
                xts[bi] = xt

            def emit_front(i):
                """W load, square, stats matmuls, mu/sigma fold for band i"""
                bd = BANDS[i]
                b = bd["b"]
                inv_k = 1.0 / (2 * b)
                xt = xts[i]
                sqr, sqc = bd["sq"]

                # W load on the SP (HWDGE) queue, a few bands ahead of use
                wpool = wps if b <= 32 else (wpb if b == 128 else wpm)
                wt = wpool.tile([bd["wrows"], bd["nwb"] * D], f16, tag="wt")
                rows = bd["wrows"] * bd["nwb"]
                wsrc = WH[bd["wr0"]:bd["wr0"] + rows, :]
                if bd["nwb"] == 1:
                    nc.sync.dma_start(out=wt[:, :], in_=wsrc)
                else:
                    nc.sync.dma_start(
                        out=wt[:, :].rearrange("p (c d) -> p c d", c=bd["nwb"]),
                        in_=wsrc.rearrange("(c p) d -> p c d", c=bd["nwb"]))
                wts[i] = wt

                x2 = x2p.tile([sqr, sqc], f16, tag="x2")
                nc.vector.tensor_mul(x2, xt[0:sqr, 0:sqc], xt[0:sqr, 0:sqc])

                xchunks = bd["xchunks"]
                last_x = len(xchunks) - 1
                pc = pssp.tile([128, 32], f32, tag="pc")
                for j in range(NJ):
                    for xi, (blk, k) in enumerate(xchunks):
                        c0 = blk * T + j * 128
                        nc.tensor.matmul(pc[:, 2 * j:2 * j + 2],
                                         xt[0:k, c0:c0 + 128],
                                         ones[0:k, :],
                                         start=(xi == 0), stop=(xi == last_x))
                for j in range(NJ):
                    for xi, (blk, k) in enumerate(xchunks):
                        c0 = blk * T + j * 128
                        nc.tensor.matmul(pc[:, 16 + 2 * j:18 + 2 * j],
                                         x2[0:k, c0:c0 + 128],
                                         ones[0:k, :],
                                         start=(xi == 0), stop=(xi == last_x))

                # batched stats processing; ms = [mu cols | sigma cols]
                ms = statp.tile([128, 16], f32, tag="ms")
                rs = statp.tile([128, NJ], f32, tag="rs")
                tmpe = statp.tile([128, NJ], f32, tag="tmpe")
                tmpm = statp.tile([128, NJ], f32, tag="tmpm")
                pcx = pc[:, 0:16].rearrange("p (a c) -> p c a", c=2)[:, 0, :]
                pcx2 = pc[:, 16:32].rearrange("p (a c) -> p c a", c=2)[:, 0, :]
                nc.vector.tensor_scalar_mul(ms[:, 0:8], pcx, inv_k)    # mu
                nc.vector.tensor_scalar_mul(tmpe, pcx2, inv_k)         # E[x^2]
                nc.vector.tensor_mul(tmpm, ms[:, 0:8], ms[:, 0:8])     # mu^2
                nc.vector.tensor_sub(tmpe, tmpe, tmpm)                 # var
                nc.scalar.activation(out=ms[:, 8:16], in_=tmpe,
                                     func=mybir.ActivationFunctionType.Sqrt,
                                     bias=epsc, scale=1.0)             # sigma
                nc.vector.reciprocal(out=rs, in_=ms[:, 8:16])          # rsqrt
                return dict(i=i, rs=rs, ms=ms)

            def emit_mid(stt):
                """mu/sigma rows via PE transpose + partition-fold DMA.
                Emitted well after front(i) so the PE transpose (which waits
                on the DVE/Act stats chain) never blocks later bands' stats
                matmuls in the in-order PE queue."""
                i, ms = stt["i"], stt["ms"]
                bd = BANDS[i]
                xt = xts[i]
                mt = psmp.tile([16, 128], f32, tag="mt")
                nc.tensor.transpose(mt, ms, ident)
                mts = statp.tile([16, 128], f16, tag="mts")
                nc.vector.tensor_scalar_mul(mts, mt, 1.0)
                mrow, mblk = bd["ms"]
                nc.sync.dma_start(
                    out=xt[mrow:mrow + 2, mblk * T:(mblk + 1) * T]
                    .rearrange("r (j p) -> r j p", j=NJ),
                    in_=mts[:, :])

            copy_acc = [0.0]
            nback = [0]

            def emit_back(stt):
                """main matmuls + scale-copy + out DMA for band stt['i']"""
                i, rs = stt["i"], stt["rs"]
                bd = BANDS[i]
                xt, wt = xts[i], wts[i]
                mains = bd["mains"]
                stage = stagep.tile([128, NJ, D], f16, tag="stage")
                for j in range(NJ):
                    po = psop.tile([128, D], f32, tag="po")
                    for ci, (xblk, wblk, K) in enumerate(mains):
                        nc.tensor.matmul(
                            po, xt[0:K, xblk * T + j * 128:xblk * T + (j + 1) * 128],
                            wt[0:K, wblk * D:(wblk + 1) * D],
                            start=(ci == 0), stop=(ci == len(mains) - 1))
                    # engine split of the PSUM->SBUF scaled copies
                    if nback[0] >= POOL_COPY_START:
                        eng = LATE_PATTERN[j]
                    else:
                        copy_acc[0] += DVE_COPY_FRAC
                        if copy_acc[0] >= 1.0:
                            copy_acc[0] -= 1.0
                            eng = "D"
                        else:
                            eng = "A"
                    if eng == "D":
                        nc.vector.tensor_scalar_mul(stage[:, j, :], po,
                                                    rs[:, j:j + 1])
                    elif eng == "P":
                        nc.gpsimd.tensor_scalar_mul(stage[:, j, :], po,
                                                    rs[:, j:j + 1])
                    else:
                        nc.scalar.activation(out=stage[:, j, :], in_=po,
                                             func=Copy, scale=rs[:, j:j + 1])
                nc.sync.dma_start(
                    out=OUT[i, :, :].rearrange("(j p) d -> p j d", p=128),
                    in_=stage)
                nback[0] += 1

            # ---- software pipeline with explicit stage lags:
            # front(idx) || mid(idx-LAG_M) || back(idx-LAG_B)
            states = []
            for idx in range(NB + LAG_B):
                mi = idx - LAG_M
                if 0 <= mi < NB:
                    emit_mid(states[mi])
                if idx < NB:
                    states.append(emit_front(ORDER[idx]))
                bi2 = idx - LAG_B
                if 0 <= bi2 < NB:
                    emit_back(states[bi2])

    nc.finalize()
    return nc


_NC = None


def kernel(X, gamma, beta, W, bias):
    global _NC
    from concourse.bass_utils import run_bass_kernel_spmd

    X = np.asarray(X, dtype=np.float32)
    gamma = np.asarray(gamma, dtype=np.float32)
    beta = np.asarray(beta, dtype=np.float32)
    W = np.asarray(W, dtype=np.float32)
    bias = np.asarray(bias, dtype=np.float32)

    Xp, w_aug = build_inputs_host(X, gamma, beta, W, bias)
    if _NC is None:
        _NC = build_nc()
    in_maps = [{"XP": Xp[b], "WA": w_aug} for b in range(NCORES)]
    res = run_bass_kernel_spmd(_NC, in_maps, list(range(NCORES))).results
    return np.stack([res[b]["OUT"] for b in range(NCORES)], axis=0).astype(
        np.float32)
